# revision 40
# baseline (speedup 1.0000x reference)
"""Causal self-attention kernel for 8 Trainium2 NeuronCores (Bass/Tile).

Problem: y = CausalSelfAttention(x) with B=4, T=2048, C=1024, 16 heads.
Sharding: 8 cores = 4 batches x 2 head-groups (8 heads each); each core
computes its batch's attention for its head group plus the partial output
projection; the host sums the two partials per batch.

v2: all-bf16 inputs, Act-engine PSUM drains, fast reciprocal normalize,
proj interleaved into the attention j-loop, PSUM ring shared yd/proj.
"""


import numpy as np
import concourse.bass as bass
import concourse.tile as tile
from concourse import mybir, bacc

F32 = mybir.dt.float32
BF16 = mybir.dt.bfloat16
EXP = mybir.ActivationFunctionType.Exp
COPY = mybir.ActivationFunctionType.Copy


def build(T=2048, HL=8, C=1024):
    """Build the per-core SPMD program. HL = heads handled by this core."""
    D = 64
    HP = HL // 2               # head pairs
    NCK = C // 128             # contraction chunks for qkv
    NI = T // 512              # 512-wide token blocks
    NTK = T // 128             # 128-wide key blocks

    nc = bacc.Bacc("TRN2", debug=False, num_devices=8)

    xt = nc.dram_tensor("xt", [NCK, 128, T], BF16, kind="ExternalInput")
    wqkv = nc.dram_tensor("wqkv", [128, HP, 3, NCK, 128], BF16,
                          kind="ExternalInput")
    wp = nc.dram_tensor("wp", [HP, 128, C], BF16, kind="ExternalInput")
    tri = nc.dram_tensor("tri", [128, 256], BF16, kind="ExternalInput")
    ident = nc.dram_tensor("ident", [128, 128], BF16, kind="ExternalInput")
    out = nc.dram_tensor("out", [T, C], BF16, kind="ExternalOutput")

    with tile.TileContext(nc) as tc:
        with tc.tile_pool(name="persist", bufs=1) as pers:
            id_sb = pers.tile([128, 128], BF16, tag="ident")
            nc.scalar.dma_start(id_sb[:], ident[:])
            tri_sb = pers.tile([128, 256], BF16, tag="tri")
            wp_sb = pers.tile([128, HP, C], BF16, tag="wp")
            q_sb = pers.tile([128, HP, T], BF16, tag="q")
            k_sb = pers.tile([128, HP, T], BF16, tag="k")
            # v^T per key block: [keys, hp, tkb, head, 64 dims + ones col]
            v_sb = pers.tile([128, HP, NTK, 2, 65], BF16, tag="v")
            y_sb = pers.tile([128, HP, T], BF16, tag="y")
            nc.vector.memset(v_sb[:, :, :, :, 64:65], 1.0)

            # ---- phase A: q^T, k^T, v for all head pairs ----
            with (
                tc.tile_pool(name="xtp", bufs=1) as xtp,
                tc.tile_pool(name="wst", bufs=3) as wst,
                tc.tile_pool(name="vtb", bufs=3) as vtp,
                tc.tile_pool(name="ps_a", bufs=6, space="PSUM") as ps_a,
                tc.tile_pool(name="ps_tr", bufs=2, space="PSUM") as ps_tr,
            ):
                xt_sb = xtp.tile([128, NCK, T], BF16, tag="xt")
                w_tiles = []
                for hp in range(HP):
                    w_h = wst.tile([128, 3, NCK, 128], BF16, tag=f"w{hp}",
                                   name=f"w{hp}")
                    # head pair 0's weights ride the Act-engine queue (idle
                    # at t=0) so they land in parallel with x on sync; later
                    # weights stay off the Act stream, which paces phase B
                    weng = nc.scalar if hp == 0 else nc.sync
                    weng.dma_start(w_h[:], wqkv[:, hp])
                    w_tiles.append(w_h)
                    if hp == 0:
                        # first 512 tokens per-chunk so the first matmul
                        # chain accumulates progressively as chunks land;
                        # the rest in one combined trigger
                        for ck in range(NCK):
                            nc.sync.dma_start(xt_sb[:, ck, 0:512],
                                              xt[ck, :, 0:512])
                        nc.sync.dma_start(tri_sb[:], tri[:])
                        nc.sync.dma_start(xt_sb[:, :, 512:T],
                                          xt[:, :, 512:T].transpose([1, 0, 2]))
                nc.scalar.dma_start(wp_sb[:], wp[:].transpose([1, 0, 2]))

                for hp in range(HP):
                    w_h = w_tiles[hp]
                    for i in range(NI):
                        ts = slice(512 * i, 512 * i + 512)
                        pq = ps_a.tile([128, 512], F32, tag="mm")
                        for ck in range(NCK):
                            nc.tensor.matmul(pq[:], w_h[:, 0, ck, :],
                                             xt_sb[:, ck, ts],
                                             start=(ck == 0),
                                             stop=(ck == NCK - 1))
                        nc.scalar.activation(q_sb[:, hp, ts], pq[:], COPY)
                        pk = ps_a.tile([128, 512], F32, tag="mm")
                        for ck in range(NCK):
                            nc.tensor.matmul(pk[:], w_h[:, 1, ck, :],
                                             xt_sb[:, ck, ts],
                                             start=(ck == 0),
                                             stop=(ck == NCK - 1))
                        nc.scalar.activation(k_sb[:, hp, ts], pk[:], COPY)
                        pv = ps_a.tile([128, 512], F32, tag="mm")
                        for ck in range(NCK):
                            nc.tensor.matmul(pv[:], w_h[:, 2, ck, :],
                                             xt_sb[:, ck, ts],
                                             start=(ck == 0),
                                             stop=(ck == NCK - 1))
                        vt_bf = vtp.tile([128, 512], BF16, tag="vt")
                        nc.scalar.activation(vt_bf[:], pv[:], COPY)
                        for f in range(4):
                            pt = ps_tr.tile([128, 128], BF16, tag="tr")
                            nc.tensor.transpose(pt[:],
                                                vt_bf[:, 128 * f:128 * f + 128],
                                                id_sb[:])
                            nc.vector.tensor_copy(
                                v_sb[:, hp, 4 * i + f, :, 0:64],
                                pt[:].rearrange("p (h d) -> p h d", h=2))

            # ---- phases B + C ----
            with (
                tc.tile_pool(name="att", bufs=20) as attp,
                tc.tile_pool(name="nrm", bufs=3) as nrm,
                tc.tile_pool(name="otp", bufs=3) as otp,
                tc.tile_pool(name="ps_s", bufs=3, space="PSUM") as ps_s,
                tc.tile_pool(name="ps_acc", bufs=1, space="PSUM") as ps_acc,
            ):
                def proj(t, tail=False):
                    ysl = slice(128 * t, 128 * t + 128)
                    po = ps_s.tile([128, 1024], F32, tag="s", name="po")
                    for ch in range(2):
                        cs = slice(512 * ch, 512 * ch + 512)
                        for hp in range(HP):
                            nc.tensor.matmul(po[:, cs], y_sb[:, hp, ysl],
                                             wp_sb[:, hp, cs],
                                             start=(hp == 0),
                                             stop=(hp == HP - 1))
                    ot = otp.tile([128, C], BF16, tag="ot", name="ot")
                    if tail and t % 2 == 1:
                        # in the tail the exp stream is finished: spread the
                        # drain copies and stores across idle engines/queues
                        nc.scalar.activation(ot[:], po[:], COPY)
                        nc.scalar.dma_start(out[ysl, :], ot[:])
                    else:
                        nc.vector.tensor_copy(ot[:], po[:])
                        nc.sync.dma_start(out[ysl, :], ot[:])

                def scores(hp, j, tkb):
                    """Emit score matmuls + exp (+ causal mask) for one
                    128-key block; returns the softmaxed att tile. Diagonal
                    blocks (r >= 0) only compute queries >= 128r."""
                    r = tkb - 4 * j
                    q0 = 128 * r if r > 0 else 0
                    tqs = slice(512 * j + q0, 512 * j + 512)
                    ks = slice(128 * tkb, 128 * tkb + 128)
                    pss = ps_s.tile([128, 1024], F32, tag="s", name="pss")
                    nc.tensor.matmul(pss[:, q0:512], k_sb[0:64, hp, ks],
                                     q_sb[0:64, hp, tqs],
                                     start=True, stop=True,
                                     tile_position=(0, 0))
                    nc.tensor.matmul(pss[:, 512 + q0:1024],
                                     k_sb[64:128, hp, ks],
                                     q_sb[64:128, hp, tqs],
                                     start=True, stop=True,
                                     tile_position=(64, 0))
                    att = attp.tile([128, 2, 512], BF16, tag="att")
                    if r < 0:
                        nc.scalar.activation(
                            att[:],
                            pss[:].rearrange("p (h t) -> p h t", h=2),
                            EXP, scale=0.125)
                    else:
                        nc.scalar.activation(
                            att[:, :, q0:512],
                            pss[:].rearrange("p (h t) -> p h t",
                                             h=2)[:, :, q0:512],
                            EXP, scale=0.125)
                        nc.vector.tensor_mul(
                            att[:, :, q0:q0 + 128],
                            att[:, :, q0:q0 + 128],
                            tri_sb[:].rearrange("p (h t) -> p h t", h=2))
                    return att

                def emit_muls(st8):
                    """Deferred normalize multiplies: emitted one hp-iteration
                    later so the yu PSUM-drain copy is never queued behind
                    them on the (in-order) vector engine."""
                    yu, dT, hp, tqs = st8
                    nc.vector.tensor_mul(y_sb[0:64, hp, tqs],
                                         yu[0:64, 0:512], dT[:, 0:512])
                    yb = nrm.tile([64, 512], BF16, tag="yb", name="yb")
                    nc.vector.tensor_mul(yb[:], yu[0:64, 512:1024],
                                         dT[:, 512:1024])
                    nc.gpsimd.dma_start(y_sb[64:128, hp, tqs], yb[:])

                pend = None
                for j in range(NI):
                    tqs = slice(512 * j, 512 * j + 512)
                    ntk = 4 * j + 4
                    for hp in range(HP):
                        pyd = ps_acc.tile([128, 1024], F32, tag="acc",
                                          name="pyd")
                        # software pipeline: scores(n+1) issue ahead of AV(n)
                        # so the PE never idles waiting for exp(n)
                        att_cur = scores(hp, j, 0)
                        for tkb in range(ntk):
                            att_nxt = (scores(hp, j, tkb + 1)
                                       if tkb + 1 < ntk else None)
                            st = (tkb == 0)
                            r = tkb - 4 * j
                            for h in range(2):
                                hb = 512 * h
                                vv = v_sb[:, hp, tkb, h, :]
                                aa = att_cur[:, h, :]
                                if r < 0:
                                    # below the diagonal: full-width, never
                                    # the last writer of any column region
                                    nc.tensor.matmul(pyd[0:65, hb:hb + 512],
                                                     vv, aa,
                                                     start=st, stop=False)
                                else:
                                    q0 = 128 * r
                                    # closing matmul for columns [q0, q0+128)
                                    nc.tensor.matmul(
                                        pyd[0:65, hb + q0:hb + q0 + 128],
                                        vv, aa[:, q0:q0 + 128],
                                        start=st, stop=True)
                                    if r < 3:
                                        # continuation for columns > q0+128
                                        nc.tensor.matmul(
                                            pyd[0:65, hb + q0 + 128:hb + 512],
                                            vv, aa[:, q0 + 128:512],
                                            start=st, stop=False)
                            att_cur = att_nxt
                        # drain PSUM fast on the Act engine (deterministic
                        # ~1.1us, right behind this hp's last exp), then
                        # normalize off the PE path
                        yu = nrm.tile([65, 1024], F32, tag="yu", name="yu")
                        nc.scalar.activation(yu[:], pyd[0:65, :], COPY)
                        den0 = nrm.tile([1, 1024], F32, tag="den0",
                                        name="den0")
                        nc.gpsimd.dma_start(den0[:], yu[64:65, :])
                        dN = nrm.tile([64, 1024], F32, tag="dN", name="dN")
                        nc.gpsimd.partition_broadcast(dN[:], den0[0:1, :])
                        dT = nrm.tile([64, 1024], F32, tag="dT", name="dT")
                        nc.vector.reciprocal_approx_fast(dT[:], dN[:])
                        if pend is not None:
                            emit_muls(pend)
                        pend = (yu, dT, hp, tqs)
                        # one projection tile of the previous block per hp
                        # iteration: each 1.7us PE burst fits inside the
                        # 2-deep exp runway, so the Act stream never starves
                        if j > 0:
                            proj(4 * (j - 1) + hp)
                emit_muls(pend)
                for t in range(4 * (NI - 1), 4 * NI):
                    proj(t, tail=True)

    nc.compile()
    return nc


def make_inputs(x_b, w_qkv, w_proj, g, HL=8):
    """Host-side prep of one core's input map.

    x_b: [T, C] fp32 (one batch), g: head-group index (0 or 1).
    """
    import ml_dtypes
    T, C = x_b.shape
    D = 64
    NCK = C // 128
    HP = HL // 2
    h0 = g * HL * D
    bf = ml_dtypes.bfloat16
    xt = np.ascontiguousarray(x_b.T.reshape(NCK, 128, T)).astype(bf)
    wqkv = np.empty((128, HP, 3, NCK, 128), dtype=np.float32)
    for kind in range(3):
        blk = w_qkv[:, kind * C + h0:kind * C + h0 + HL * D]
        wqkv[:, :, kind] = blk.reshape(NCK, 128, HP, 128).transpose(1, 2, 0, 3)
    wqkv = np.ascontiguousarray(wqkv).astype(bf)
    wpz = np.ascontiguousarray(
        w_proj[h0:h0 + HL * D, :].reshape(HP, 128, C)).astype(bf)
    t1 = np.triu(np.ones((128, 128), dtype=np.float32))
    tri = np.concatenate([t1, t1], axis=1).astype(bf)
    ident = np.eye(128, dtype=np.float32).astype(bf)
    return {"xt": xt, "wqkv": wqkv, "wp": wpz, "tri": tri, "ident": ident}


_NC_CACHE = {}


def kernel(x, w_qkv, w_proj):
    import numpy as np
    from concourse.bass_utils import run_bass_kernel_spmd

    x = np.ascontiguousarray(np.asarray(x, dtype=np.float32))
    w_qkv = np.ascontiguousarray(np.asarray(w_qkv, dtype=np.float32))
    w_proj = np.ascontiguousarray(np.asarray(w_proj, dtype=np.float32))
    B, T, C = x.shape

    key = (T, C)
    if key not in _NC_CACHE:
        _NC_CACHE[key] = build(T=T, HL=8, C=C)
    nc = _NC_CACHE[key]

    in_maps = [make_inputs(x[c // 2], w_qkv, w_proj, c % 2, HL=8)
               for c in range(8)]
    res = run_bass_kernel_spmd(nc, in_maps, core_ids=list(range(8)),
                               trace=False)

    out = np.zeros((B, T, C), dtype=np.float32)
    for c in range(8):
        out[c // 2] += np.asarray(res.results[c]["out"], dtype=np.float32)
    return out
